# revision 1
# baseline (speedup 1.0000x reference)
"""CubeAttention Trainium2 Bass kernel (8-core SPMD), v2.

Data-parallel over the query grid: the 20^3 grid splits into 8 slabs of
[5,10,20] (4 blocks along i x 2 halves along j). Each core gets a haloed
bf16 slab (channel-major, ones-row appended for fused bias; shipped twice,
(i,j,k)- and (k,j,i)-raster) plus one packed constant tile; host reassembles.

v2 vs v1:
  - all matmuls bf16 (4x PE throughput), fp32 PSUM accumulate.
  - logits computed TRANSPOSED per si-plane ([81 support, 125 queries]):
    the exp activation is the one PSUM->SBUF pass, no S transposes.
  - softmax normalisation deferred past the output projection (per-query
    1/Z on the [125,64] result); Z comes free from a ones-column appended
    to the s-major value table (psv row 64).
  - axis-marginals for the relpos-value fixup via a constant indicator
    matmul into [73,125] PSUM with groups at partitions 0/32/64 so the 15
    fixup matmuls read them directly (no DMA bounce).
  - s-major value table built once ([3024,65] DRAM, (k,j,i)-raster), one
    3-dim gather DMA per block replaces stage+9 transposes+9 copies.
  - 3 DMAs per block total (vp gather, 2 C-row moves), split between the
    HWDGE (SP) and SWDGE (Pool) paths; output accumulated in SBUF and
    stored with a single DMA at the end.
"""

import numpy as np

SCOPE, GN, D, CAP = 2, 20, 64, 32
NEG = np.float32(-1e9)

# wpack column layout (bf16, [128, _WCOLS])
_O_WQ, _O_WK, _O_WV, _O_WO = 0, 64, 128, 192
_O_G = 256                 # [64, 15*9]  shifted-G lhsT tiles (x,g)
_O_RV = _O_G + 135         # [9, 15*64]  RVSH(x,g), rows at 32*x
_O_MASK = _O_RV + 960      # [9, 8*375]  masks, cols (blk, x, q)
_O_MARG = _O_MASK + 3000   # [81, 9*73]  marginal indicator lhsT per si
_O_IND = _O_MARG + 657     # [27, 3*729] ind rows for kp (3 ping-pong)
_WCOLS = _O_IND + 2187

_CACHE = {}


def _bass_mod():
    if "nc" in _CACHE:
        return _CACHE["nc"]
    import sys
    for p in ("/opt/trn_rl_repo", "/root/.axon_site/_ro/trn_rl_repo"):
        if p not in sys.path:
            sys.path.append(p)
    import concourse.tile as tile
    from concourse import bacc, mybir

    f32 = mybir.dt.float32
    bf16 = mybir.dt.bfloat16
    AF = mybir.ActivationFunctionType

    nc = bacc.Bacc("TRN2", target_bir_lowering=False, debug=False)
    P = {}
    P["seT"] = nc.declare_dram_parameter("seT", [65, 3024], bf16, isOutput=False)
    P["seTk"] = nc.declare_dram_parameter("seTk", [65, 3024], bf16, isOutput=False)
    P["wpack"] = nc.declare_dram_parameter("wpack", [128, _WCOLS], bf16,
                                           isOutput=False)
    P["bob"] = nc.declare_dram_parameter("bob", [125, 384], f32, isOutput=False)
    out_p = nc.declare_dram_parameter("out", [8, 125, 64], f32, isOutput=True)

    with tile.TileContext(nc) as tc:
        with (
            tc.tile_pool(name="const", bufs=1) as const,
            tc.tile_pool(name="sS", bufs=3) as sS,
            tc.tile_pool(name="sVP", bufs=3) as sVP,
            tc.tile_pool(name="sQA", bufs=3) as sQA,
            tc.tile_pool(name="sC", bufs=3) as sC,
            tc.tile_pool(name="sW", bufs=2) as sW,
            tc.tile_pool(name="dram", bufs=1, space="DRAM") as dpool,
            tc.tile_pool(name="psL", bufs=4, space="PSUM") as psL,
            tc.tile_pool(name="psS", bufs=2, space="PSUM") as psS,
            tc.tile_pool(name="psV", bufs=1, space="PSUM") as psV,
            tc.tile_pool(name="psM", bufs=1, space="PSUM") as psM,
        ):
            # ---- constants ----
            wp = const.tile([128, _O_IND], bf16, tag="wp")
            nc.sync.dma_start(wp[:], P["wpack"][:, 0:_O_IND])
            seT = const.tile([65, 3024], bf16, tag="seT")
            nc.sync.dma_start(seT[:], P["seT"][:])
            seT4 = seT[:].rearrange("p (i j k) -> p i j k", i=9, j=14, k=24)
            seTk = const.tile([65, 3024], bf16, tag="seTk")
            nc.sync.dma_start(seTk[:], P["seTk"][:])
            bobf = const.tile([125, 384], f32, tag="bobf")
            nc.sync.dma_start(bobf[:], P["bob"][:])
            identf = const.tile([1, 1], f32, tag="identf")
            nc.vector.memset(identf[:], 1.0)

            # kp: [91, 3*729]; rows 0:64 KP (per block), 64:91 ind27 (const)
            kp = const.tile([91, 3 * 729], bf16, tag="kp")
            nc.sync.dma_start(kp[64:91, :], P["wpack"][0:27, _O_IND:_WCOLS])

            # ---- projections (bias via ones-row x bias-row) ----
            KPT = const.tile([64, 3024], bf16, tag="KPT")
            for c in range(6):
                sl = slice(504 * c, 504 * (c + 1))
                ps = psS.tile([64, 504], f32, tag="x")
                nc.tensor.matmul(ps[:], wp[0:65, _O_WK:_O_WK + 64],
                                 seT[:, sl], start=True, stop=True)
                if c % 2 == 0:
                    nc.scalar.copy(KPT[:, sl], ps[:])
                else:
                    nc.vector.tensor_copy(KPT[:, sl], ps[:])
            KPT4 = KPT[:].rearrange("p (i j k) -> p i j k", i=9, j=14, k=24)

            # s-major value table, (k,j,i)-raster rows, 65th col = ones
            vsm = const.tile([126, 24 * 65], bf16, tag="vsm")
            nc.gpsimd.memset(vsm[:], 1.0)
            for c in range(24):
                ps = psS.tile([126, 64], f32, tag="x")
                nc.tensor.matmul(ps[:], seTk[:, 126 * c:126 * (c + 1)],
                                 wp[0:65, _O_WV:_O_WV + 64],
                                 start=True, stop=True)
                if c % 2 == 0:
                    nc.vector.tensor_copy(vsm[:, 65 * c:65 * c + 64], ps[:])
                else:
                    nc.scalar.copy(vsm[:, 65 * c:65 * c + 64], ps[:])
            vpsm = dpool.tile([3024, 65], bf16, tag="vpsm")
            nc.sync.dma_start(
                vpsm[:].rearrange("(c p) e -> p c e", c=24),
                vsm[:].rearrange("p (c e) -> p c e", c=24))
            # gather view: row (k*126 + j*9 + i), (i ch) merged contiguous
            vpj = vpsm[:].rearrange("(k j i) ch -> j k (i ch)", k=24, j=14)

            Qall = const.tile([64, 1000], bf16, tag="Qall")
            for i in range(5):
                ps = psS.tile([64, 200], f32, tag="x")
                nc.tensor.matmul(ps[:], wp[0:65, _O_WQ:_O_WQ + 64],
                                 seT4[:, i + 2, 2:12, 2:22],
                                 start=True, stop=True)
                if i % 2 == 0:
                    nc.vector.tensor_copy(Qall[:, 200 * i:200 * (i + 1)], ps[:])
                else:
                    nc.scalar.copy(Qall[:, 200 * i:200 * (i + 1)], ps[:])
            Qall4 = Qall[:].rearrange("p (i j k) -> p i j k", i=5, j=10, k=20)

            # ---- per-block stages ----
            def stage_A(blk):
                bj, bkk = blk // 4, blk % 4
                pp = blk % 3
                jsl = slice(5 * bj, 5 * bj + 9)
                ksl = slice(5 * bkk, 5 * bkk + 9)

                nc.vector.tensor_copy(
                    kp[0:64, 729 * pp:729 * (pp + 1)].rearrange(
                        "p (i a c) -> p i a c", i=9, a=9),
                    KPT4[:, :, jsl, ksl])

                vp = sVP.tile([81, 9 * 65], bf16, tag="vp")
                nc.sync.dma_start(vp[:], vpj[jsl, ksl, :])

                qa = sQA.tile([91, 125], bf16, tag="qa")
                nc.vector.tensor_copy(
                    qa[0:64, :].rearrange("p (a b c) -> p a b c", a=5, b=5),
                    Qall4[:, :, 5 * bj:5 * bj + 5, 5 * bkk:5 * bkk + 5])
                qa3 = qa[:].rearrange("p (a b c) -> p a b c", a=5, b=5)

                # C rows: 15 shifted-G matmuls into one [9,375] PSUM tile
                psC = psL.tile([9, 375], f32, tag="L")
                psC5 = psC[:].rearrange("p (x a b c) -> p x a b c",
                                        x=3, a=5, b=5)
                for xi in range(3):
                    for g in range(5):
                        lhsT = wp[0:64, _O_G + 9 * (5 * xi + g):
                                  _O_G + 9 * (5 * xi + g) + 9]
                        if xi == 0:
                            rhs, o = qa3[0:64, g, :, :], psC5[:, 0, g, :, :]
                        elif xi == 1:
                            rhs, o = qa3[0:64, :, g, :], psC5[:, 1, :, g, :]
                        else:
                            rhs, o = qa3[0:64, :, :, g], psC5[:, 2, :, :, g]
                        nc.tensor.matmul(o, lhsT, rhs, start=True, stop=True)
                csb = sC.tile([9, 375], bf16, tag="csb")
                nc.vector.tensor_add(
                    csb[:], psC[:],
                    wp[0:9, _O_MASK + 375 * blk:_O_MASK + 375 * (blk + 1)])
                nc.vector.tensor_copy(qa[64:73, :], csb[:, 0:125])
                nc.sync.dma_start(qa[73:82, :], csb[:, 125:250])
                nc.gpsimd.dma_start(qa[82:91, :], csb[:, 250:375])
                return pp, vp, qa

            def stage_QKT(blk, st):
                pp, vp, qa = st
                Ss = []
                for g in range(3):
                    pl = psL.tile([81, 375], f32, tag="L")
                    for si in range(3 * g, 3 * g + 3):
                        nc.tensor.matmul(
                            pl[:, 125 * (si % 3):125 * (si % 3) + 125],
                            kp[:, 729 * pp + 81 * si:729 * pp + 81 * si + 81],
                            qa[:], start=True, stop=True)
                    Sg = sS.tile([81, 375], bf16, tag="S")
                    nc.scalar.activation(Sg[:], pl[:], AF.Exp)
                    Ss.append(Sg)
                return Ss

            import os as _os
            _FIXMM = _os.environ.get("KFIXMM", "1") == "1"
            _KZT = _os.environ.get("KZT", "1") == "1"

            def stage_AVM(blk, st, Ss):
                pp, vp, qa = st
                psv = psV.tile([65, 125], f32, tag="v")
                psm = psM.tile([73, 125], f32, tag="m")
                for si in range(9):
                    Ssl = Ss[si // 3][:, 125 * (si % 3):125 * (si % 3) + 125]
                    nc.tensor.matmul(psv[:], vp[:, 65 * si:65 * si + 65],
                                     Ssl, start=(si == 0),
                                     stop=(si == 8 and not _FIXMM))
                    nc.tensor.matmul(
                        psm[:],
                        wp[0:81, _O_MARG + 73 * si:_O_MARG + 73 * si + 73],
                        Ssl, start=(si == 0), stop=(si == 8))
                msb = sW.tile([41, 125], bf16, tag="msb")
                nc.vector.tensor_copy(msb[:], psm[0:41, :])
                msbKf = sW.tile([9, 125], f32, tag="msbKf")
                nc.vector.tensor_copy(msbKf[:], psm[64:73, :])
                Zsb = sW.tile([1, 125], f32, tag="Zsb")
                nc.scalar.copy(Zsb[:], psv[64:65, :])
                return psv, msb, msbKf, Zsb

            def stage_FIX(blk, avm):
                psv, msb, msbKf, Zsb = avm
                psv3 = psv[0:64, :].rearrange("p (a b c) -> p a b c", a=5, b=5)
                msb3 = msb[:].rearrange("p (a b c) -> p a b c", a=5, b=5)
                # axis-k goes to its own PSUM tile/accumulation group: its
                # stride-5-inner operands abort on HW when mixed into the
                # same group as the contiguous-inner i/j matmuls
                psvK = psS.tile([64, 125], f32, tag="x")
                psvK3 = psvK[:].rearrange("p (a b c) -> p a b c", a=5, b=5)
                for xi in range(3 if _FIXMM else 0):
                    r0 = 32 * xi
                    for g in range(5):
                        lhsT = wp[r0:r0 + 9, _O_RV + 64 * (5 * xi + g):
                                  _O_RV + 64 * (5 * xi + g) + 64]
                        if xi == 0:
                            rhs, o = msb3[0:9, g, :, :], psv3[:, g, :, :]
                            nc.tensor.matmul(o, lhsT, rhs, start=False,
                                             stop=False)
                        elif xi == 1:
                            rhs, o = msb3[32:41, :, g, :], psv3[:, :, g, :]
                            nc.tensor.matmul(o, lhsT, rhs, start=False,
                                             stop=(g == 4))
                        else:
                            # stride-5-inner operands only work in fp32
                            # (bf16 aborts the exec unit); disjoint slices
                            lhsT = bobf[0:9, 64 + 64 * g:128 + 64 * g]
                            rhs = msbKf[:].rearrange(
                                "p (a b c) -> p a b c", a=5, b=5)[:, :, :, g]
                            o = psvK3[:, :, :, g]
                            nc.tensor.matmul(o, lhsT, rhs, start=True,
                                             stop=True)
                avf = sW.tile([64, 125], bf16, tag="avf")
                if _FIXMM:
                    ksb = sW.tile([64, 125], bf16, tag="ksb")
                    nc.scalar.copy(ksb[:], psvK[:])
                    nc.vector.tensor_add(avf[:], psv[0:64, :], ksb[:])
                else:
                    nc.vector.tensor_copy(avf[:], psv[0:64, :])
                rzt = None
                if _KZT:
                    psr = psS.tile([125, 1], f32, tag="x")
                    nc.tensor.transpose(psr[:], Zsb[:], identf[:])
                    rzt = sW.tile([125, 1], f32, tag="rzt")
                    nc.vector.reciprocal(rzt[:], psr[:])
                return avf, rzt

            osb = const.tile([125, 8 * 64], f32, tag="osb")

            def stage_FIN(blk, fx):
                avf, rzt = fx
                pso = psS.tile([125, 64], f32, tag="x")
                nc.tensor.matmul(pso[:], avf[:], wp[0:64, _O_WO:_O_WO + 64],
                                 start=True, stop=True)
                osl = osb[:, 64 * blk:64 * (blk + 1)]
                if rzt is not None:
                    nc.scalar.activation(osl, pso[:], AF.Identity, scale=rzt[:])
                else:
                    nc.scalar.copy(osl, pso[:])
                nc.vector.tensor_add(osl, osl, bobf[:, 0:64])

            import os
            _NB = int(os.environ.get("KNB", "8"))
            _ST = int(os.environ.get("KST", "4"))  # 1=A,2=+QKT,3=+AVM,4=all
            sts = {0: stage_A(0)}
            if _NB > 1:
                sts[1] = stage_A(1)
            fixes = {}
            for n in range(_NB):
                if n + 2 < _NB:
                    sts[n + 2] = stage_A(n + 2)
                if _ST < 2:
                    continue
                Ss = stage_QKT(n, sts[n])
                if n > 0 and (n - 1) in fixes:
                    stage_FIN(n - 1, fixes[n - 1])
                if _ST < 3:
                    continue
                avm = stage_AVM(n, sts[n], Ss)
                if _ST < 4:
                    continue
                fixes[n] = stage_FIX(n, avm)
            if 7 in fixes:
                stage_FIN(7, fixes[7])
            if _ST < 4:  # ensure osb written so out DMA is valid
                nc.vector.memset(osb[:], 0.0)

            # keep the SBUF partition dim outermost in the enumeration
            nc.sync.dma_start(
                out_p[:].rearrange("b q c -> q b c"), osb[:])

    nc.compile()
    _CACHE["nc"] = nc
    return nc


def _masks_for_core(bi, h):
    q = np.arange(125)
    a, b, c = q // 25, (q // 5) % 5, q % 5
    sig = np.arange(9)[:, None]

    def vmask(qx, off):
        return (qx + off > 2) & (qx + off < 22)

    out = np.zeros((8, 3, 9, 125), np.float32)
    for blk in range(8):
        bj, bkk = blk // 4, blk % 4
        qi = 5 * bi + a
        qj = 10 * h + 5 * bj + b
        qk = 5 * bkk + c
        oi = sig - a[None, :]
        oj = sig - b[None, :]
        ok = sig - c[None, :]
        wi = (oi >= 0) & (oi <= 4)
        wj = (oj >= 0) & (oj <= 4)
        wk = (ok >= 0) & (ok <= 4)
        out[blk, 0] = np.where(wi & vmask(qj[None, :], oi), 0.0, NEG)
        out[blk, 1] = np.where(wj & vmask(qi[None, :], oj), 0.0, NEG)
        out[blk, 2] = np.where(wk & vmask(qk[None, :], ok), 0.0, NEG)
    return out


def _bf16(x):
    import ml_dtypes
    return np.asarray(x, np.float32).astype(ml_dtypes.bfloat16)


def _pack_weights(inputs, bi, h):
    relpos = np.asarray(inputs["relpos_w"], np.float32)
    Wk = np.asarray(inputs["Wk"], np.float32)
    Wv = np.asarray(inputs["Wv"], np.float32)
    wpf = np.zeros((128, _WCOLS), np.float32)

    wpf[0:64, _O_WQ:_O_WQ + 64] = inputs["Wq"]
    wpf[64, _O_WQ:_O_WQ + 64] = inputs["bq"]
    wpf[0:64, _O_WK:_O_WK + 64] = Wk[96:160]
    wpf[64, _O_WK:_O_WK + 64] = inputs["bk"]
    wpf[0:64, _O_WV:_O_WV + 64] = Wv[96:160]
    wpf[64, _O_WV:_O_WV + 64] = inputs["bv"]
    wpf[0:64, _O_WO:_O_WO + 64] = inputs["Wo"]

    # shifted-G lhsT tiles; axis i<-Wk[32:64], j<-Wk[0:32], k<-Wk[64:96]
    G = [relpos @ Wk[32:64], relpos @ Wk[0:32], relpos @ Wk[64:96]]
    for x in range(3):
        Zk = np.zeros((64, 13), np.float32)
        Zk[:, 4:9] = G[x].T
        for g in range(5):
            wpf[0:64, _O_G + 9 * (5 * x + g):_O_G + 9 * (5 * x + g) + 9] = (
                Zk[:, 4 - g:13 - g])

    RVs = [relpos @ Wv[32:64], relpos @ Wv[0:32], relpos @ Wv[64:96]]
    for x in range(3):
        r0 = 32 * x  # base partition must match the fixup rhs msb group
        for g in range(5):
            t = np.zeros((9, 64), np.float32)
            t[g:g + 5] = RVs[x]
            wpf[r0:r0 + 9,
                _O_RV + 64 * (5 * x + g):_O_RV + 64 * (5 * x + g) + 64] = t

    masks = _masks_for_core(bi, h)  # [8,3,9,125]
    wpf[0:9, _O_MASK:_O_MASK + 3000] = (
        masks.transpose(2, 0, 1, 3).reshape(9, 3000))

    p81 = np.arange(81)
    pa, pc = p81 // 9, p81 % 9
    marg = np.zeros((81, 9, 73), np.float32)
    for si in range(9):
        marg[:, si, si] = 1.0
        for t in range(9):
            marg[:, si, 32 + t] = (pa == t)
            marg[:, si, 64 + t] = (pc == t)
    wpf[0:81, _O_MARG:_O_MARG + 657] = marg.reshape(81, 657)

    s729 = np.arange(729)
    s_i, s_a, s_c = s729 // 81, (s729 // 9) % 9, s729 % 9
    ind27 = np.zeros((27, 729), np.float32)
    for t in range(9):
        ind27[t] = (s_i == t)
        ind27[9 + t] = (s_a == t)
        ind27[18 + t] = (s_c == t)
    wpf[0:27, _O_IND:_WCOLS] = np.tile(ind27, (1, 3))
    return _bf16(wpf)


def _make_in_maps(inputs):
    se = np.asarray(inputs["spatial_embeddings"], np.float32)
    inputs = {k: np.asarray(v, np.float32) for k, v in inputs.items()}
    se_pad = np.pad(se, ((2, 2),) * 3 + ((0, 0),))
    relpos = inputs["relpos_w"]
    Wv = inputs["Wv"]
    fconst = np.zeros((125, 384), np.float32)
    fconst[:, 0:64] = np.broadcast_to(inputs["bo"], (125, 64))
    RVk = relpos @ Wv[64:96]
    for g in range(5):
        t = np.zeros((9, 64), np.float32)
        t[g:g + 5] = RVk
        fconst[0:9, 64 + 64 * g:128 + 64 * g] = t
    bob = fconst
    in_maps = []
    for core in range(8):
        bi, h = core // 2, core % 2
        slab = se_pad[5 * bi:5 * bi + 9, 10 * h:10 * h + 14, :, :]
        seT65 = np.ones((65, 3024), np.float32)
        seT65[0:64] = slab.transpose(3, 0, 1, 2).reshape(64, 3024)
        seTk65 = np.ones((65, 3024), np.float32)
        seTk65[0:64] = slab.transpose(3, 2, 1, 0).reshape(64, 3024)
        m = dict(seT=_bf16(seT65), seTk=_bf16(seTk65),
                 wpack=_pack_weights(inputs, bi, h), bob=bob)
        in_maps.append(m)
    return in_maps


def _assemble(results):
    out = np.empty((20, 20, 20, 64), np.float32)
    for core in range(8):
        bi, h = core // 2, core % 2
        blocks = np.asarray(results[core]["out"]).reshape(8, 5, 5, 5, 64)
        for blk in range(8):
            bj, bkk = blk // 4, blk % 4
            out[5 * bi:5 * bi + 5,
                10 * h + 5 * bj:10 * h + 5 * bj + 5,
                5 * bkk:5 * bkk + 5] = blocks[blk]
    return out


def kernel(**inputs):
    import sys
    for pth in ("/opt/trn_rl_repo", "/root/.axon_site/_ro/trn_rl_repo"):
        if pth not in sys.path:
            sys.path.append(pth)
    from concourse.bass_utils import run_bass_kernel_spmd

    nc = _bass_mod()
    in_maps = _make_in_maps(inputs)
    res = run_bass_kernel_spmd(nc, in_maps, core_ids=list(range(8)))
    return _assemble(res.results)

